# revision 12
# baseline (speedup 1.0000x reference)
"""Trainium2 Bass kernel for the 15-qubit, 4-layer variational circuit.

Problem: batch of 32 circuit evaluations; each evolves a 2^15 state through
4 layers of (RY RZ RX RZ RX per qubit + CNOT chain) and measures <Z...Z>.

Strategy (8 NeuronCores, batch-parallel, zero collectives):
  - Each core simulates 4 batch elements; the full state (4 x 32768 complex64
    as separate f32 re/im planes) lives in SBUF the whole time.
  - State layout per batch: S[p, f], p = 7 "partition qubits" (6,0,1,2,3,4,5
    MSB->LSB), f = 8 "free qubits" (7..14 MSB->LSB).
  - Per layer, two PE matmul stages, each of which applies a dense fused gate
    matrix AND transposes the layout (out = lhsT.T @ rhs with the state as the
    stationary operand):
      stage A: A = C_P @ kron(G_q for partition qubits)   (128x128 complex)
      stage B: K = C_F @ kron(G_q for free qubits)        (256x256 complex)
    where G_q = RX RZ RX RZ(x2) RY(x1) is the per-qubit fused 1q gate and
    C_P/C_F are the intra-group CNOT chains folded in host-side.
  - The one straddling CNOT(6,7) conjugated past C_F becomes
    "if q6(p): f ^= 0xFF" (free-index reversal), folded into the PSUM->SBUF
    eviction copy after stage B at zero cost. On the last layer it is skipped
    entirely (XOR by 0xFF has even parity, so the Z...Z sign is unchanged).
  - Real/imag cross terms are packed side by side in the moving operand so
    every matmul has free dim >= 256, where float32r runs at full PE rate.
  - Finale: ACT squares |amp|^2 straight out of PSUM, DVE applies the
    (-1)^popcount sign tile, ACT row-reduces via accum_out, one tiny matmul
    reduces over partitions, and a [1,4] DMA returns the 4 expectations.
"""

import sys

if "/opt/trn_rl_repo" not in sys.path:
    sys.path.append("/opt/trn_rl_repo")

import numpy as np

N_QUBITS = 15
N_LAYERS = 4
BATCH = 32
DIM = 1 << N_QUBITS
N_CORES = 8
NB = BATCH // N_CORES  # batches per core

PART_QUBITS = [6, 0, 1, 2, 3, 4, 5]       # p bit MSB->LSB (q6 = p MSB)
FREE_QUBITS = [7, 8, 9, 10, 11, 12, 13, 14]  # f bit MSB->LSB

A_DT = "f16"   # stage-A matmul dtype: f32r | f16 | bf16 | f32
B_DT = "f16"   # stage-B matmul dtype (merged DMA when equal to A_DT)
NEG_ON_POOL = True

# ----------------------------------------------------------------- host math


def _rx(t):
    c, s = np.cos(t / 2), -1j * np.sin(t / 2)
    return np.array([[c, s], [s, c]], dtype=np.complex128)


def _ry(t):
    c, s = np.cos(t / 2), np.sin(t / 2)
    return np.array([[c, -s], [s, c]], dtype=np.complex128)


def _rz(t):
    return np.array(
        [[np.exp(-1j * t / 2), 0], [0, np.exp(1j * t / 2)]], dtype=np.complex128
    )


def _chain_perm(qubit_list, nbits, bitpos):
    """perm[old] = new index after CNOT(q, q+1) for q in qubit_list."""
    idx = np.arange(1 << nbits)
    bits = {q: (idx >> (nbits - 1 - pos)) & 1 for q, pos in bitpos.items()}
    for q in qubit_list:
        bits[q + 1] = bits[q + 1] ^ bits[q]
    new = np.zeros(1 << nbits, dtype=np.int64)
    for q, pos in bitpos.items():
        new |= bits[q] << (nbits - 1 - pos)
    return new


def _kron_list(mats):
    out = np.array([[1.0 + 0j]])
    for m in mats:
        out = np.kron(out, m)
    return out


_P_BITPOS = {q: i for i, q in enumerate(PART_QUBITS)}
_F_BITPOS = {q: i for i, q in enumerate(FREE_QUBITS)}
_PERM_P = _chain_perm(range(0, 6), 7, _P_BITPOS)
_PERM_F = _chain_perm(range(7, 14), 8, _F_BITPOS)


def _stage_matrices(x_b, thetas):
    x1 = np.arcsin(np.float64(x_b))
    x2 = np.arccos(np.float64(x_b) ** 2)
    E = _rz(x2) @ _ry(x1)
    As, Ks = [], []
    for l in range(N_LAYERS):
        G = {}
        for q in range(N_QUBITS):
            th = thetas[l, q].astype(np.float64)
            G[q] = _rx(th[2]) @ _rz(th[1]) @ _rx(th[0]) @ E
        kp = _kron_list([G[q] for q in PART_QUBITS])
        A = np.zeros_like(kp)
        A[_PERM_P, :] = kp
        kf = _kron_list([G[q] for q in FREE_QUBITS])
        K = np.zeros_like(kf)
        K[_PERM_F, :] = kf
        if l == N_LAYERS - 1:
            # regroup output index by popcount parity: j -> (parity, j & 127),
            # so the finale reduces sign-homogeneous contiguous blocks
            j = np.arange(256)
            par = np.array([bin(v).count("1") & 1 for v in j])
            jprime = (par << 7) | (j & 127)
            K2 = np.zeros_like(K)
            K2[jprime, :] = K
            K = K2
        As.append(A)
        Ks.append(K)
    return As, Ks


def _sign_tile():
    pc = lambda v: np.array([bin(int(i)).count("1") for i in v])
    sp = 1.0 - 2.0 * (pc(np.arange(128)) % 2)
    sf = 1.0 - 2.0 * (pc(np.arange(256)) % 2)
    return (sp[:, None] * sf[None, :]).astype(np.float32)


def _host_inputs_for_core(x_core, thetas):
    """MA [NB*4, 128, 384] and MB [NB*4, 2, 128, 768] f32 for one core."""
    ma = np.zeros((NB * N_LAYERS, 128, 256), dtype=np.float32)
    mb = np.zeros((NB * N_LAYERS, 128, 1024), dtype=np.float32)
    # [Ai | Ar | Ki0 | Kr0 | Ki1 | Kr1] per (b, l)
    for b in range(NB):
        As, Ks = _stage_matrices(x_core[b], thetas)
        for l in range(N_LAYERS):
            A = As[l]
            ArT = np.ascontiguousarray(A.real.T).astype(np.float32)
            AiT = np.ascontiguousarray(A.imag.T).astype(np.float32)
            ma[b * N_LAYERS + l] = np.concatenate([AiT, ArT], axis=1)
            KT = Ks[l].T
            KTr = KT.real.astype(np.float32)
            KTi = KT.imag.astype(np.float32)
            mb[b * N_LAYERS + l] = np.concatenate(
                [KTi[0:128], KTr[0:128], KTi[128:256], KTr[128:256]], axis=1)
    return ma, mb


# -------------------------------------------------------------- device build

_CACHE = {}


def _build_module():
    import concourse.bacc as bacc
    import concourse.mybir as mybir
    import concourse.tile as tile

    f32 = mybir.dt.float32
    dts = {"f32r": mybir.dt.float32r, "bf16": mybir.dt.bfloat16,
           "f16": mybir.dt.float16, "f32": mybir.dt.float32}
    dta = dts[A_DT]
    dtb = dts[B_DT]
    merged = A_DT == B_DT
    Square = mybir.ActivationFunctionType.Square
    Copy = mybir.ActivationFunctionType.Copy

    nc = bacc.Bacc("TRN2", target_bir_lowering=False, debug=False)
    if merged:
        mab_d = nc.dram_tensor("mab", [NB * N_LAYERS, 128, 1280], dta,
                               kind="ExternalInput")
    else:
        ma_d = nc.dram_tensor("ma", [NB * N_LAYERS, 128, 256], dta,
                              kind="ExternalInput")
        mb_d = nc.dram_tensor("mb", [NB * N_LAYERS, 128, 1024], dtb,
                              kind="ExternalInput")
    sp_d = nc.dram_tensor("spv", [128, 2], f32, kind="ExternalInput")
    one_d = nc.dram_tensor("one", [1, NB], dta, kind="ExternalInput")
    res_d = nc.dram_tensor("res", [1, NB], f32, kind="ExternalOutput")

    neg = (lambda o, i: nc.gpsimd.tensor_scalar_mul(o, i, -1.0)) \
        if NEG_ON_POOL else \
        (lambda o, i: nc.vector.tensor_scalar_mul(o, i, -1.0))

    with tile.TileContext(nc) as tc:
        with tc.tile_pool(name="state", bufs=1) as stp, \
             tc.tile_pool(name="xbuf", bufs=3) as xp, \
             tc.tile_pool(name="mats", bufs=3) as mp, \
             tc.tile_pool(name="fin", bufs=2) as fp, \
             tc.tile_pool(name="misc", bufs=1) as msc, \
             tc.tile_pool(name="pa", bufs=2, space="PSUM") as pa, \
             tc.tile_pool(name="pb", bufs=2, space="PSUM") as pb, \
             tc.tile_pool(name="pf", bufs=1, space="PSUM") as pf:

            s_r = stp.tile([128, 256 * NB], dta, tag="sr", name="sr")
            s_i = stp.tile([128, 256 * NB], dta, tag="si", name="si")
            spv = msc.tile([128, 2], f32, tag="spv")
            rows = msc.tile([128, 4 * NB], f32, tag="rows")
            res_s = msc.tile([1, NB], f32, tag="res")
            ztmp = msc.tile([128, 256 * NB], f32, tag="ztmp")

            nc.sync.dma_start(spv[:], sp_d[:])
            nc.vector.memset(rows[:], 0.0)
            nc.vector.memset(ztmp[:], 0.0)
            nc.vector.tensor_copy(s_r[:], ztmp[:])
            nc.scalar.copy(s_i[:], ztmp[:])
            nc.sync.dma_start(s_r[0:1, 0 : 256 * NB : 256], one_d[:])

            for l in range(N_LAYERS):
                for b in range(NB):
                    i_bl = b * N_LAYERS + l
                    # MT cols: [Ai 0:128 | Ar 128:256 | Ki0 256:512 |
                    #   Kr0 512:768 | Ki1 768:1024 | Kr1 1024:1280 |
                    #   nAi 1280:1408 | nKi0 1408:1664 | nKi1 1664:1920 | pad]
                    if merged:
                        mt = mp.tile([128, 2432], dta, tag="mt")
                        nc.sync.dma_start(mt[:, 0:1280], mab_d[i_bl])
                        mtb = mt[:, 256:1280]
                    else:
                        mt = mp.tile([128, 2432], dtb, tag="mt")
                        mta_t = mp.tile([128, 256], dta, tag="mta")
                        nc.sync.dma_start(mta_t[:], ma_d[i_bl])
                        nc.sync.dma_start(mt[:, 256:1280], mb_d[i_bl])
                    neg(mt[:, 1408:1664], mt[:, 256:512])
                    neg(mt[:, 1664:1920], mt[:, 768:1024])
                    if merged:
                        neg(mt[:, 1280:1408], mt[:, 0:128])
                        rhs_ar = mt[:, 0:256]
                        rhs_ai = mt[:, 128:2432].rearrange(
                            "u (a v) -> u a v", v=1152)[:, :, 0:128]
                    else:
                        mta_n = mp.tile([128, 384], dta, tag="mtan")
                        nc.vector.tensor_copy(mta_n[:, 0:256], mta_t[:])
                        neg(mta_n[:, 256:384], mta_t[:, 0:128])
                        rhs_ar = mta_n[:, 0:256]
                        rhs_ai = mta_n[:, 128:384]

                    sb = slice(b * 256, (b + 1) * 256)
                    ps_a = pa.tile([128, 512], f32, tag="pa")
                    for h in (0, 1):
                        sl = slice(b * 256 + h * 128, b * 256 + (h + 1) * 128)
                        po = slice(h * 256, (h + 1) * 256)
                        nc.tensor.matmul(ps_a[:, po], s_r[:, sl], rhs_ar,
                                         start=(h == 0), stop=False)
                        nc.tensor.matmul(ps_a[:, po], s_i[:, sl], rhs_ai,
                                         start=False, stop=(h == 1))
                    xr = xp.tile([128, 256], dtb, tag="xr")
                    xi = xp.tile([128, 256], dtb, tag="xi")
                    pav = ps_a[:].rearrange("u (h c p) -> u h c p", c=2, p=128)
                    nc.scalar.copy(
                        xi[:].rearrange("u (h p) -> u h p", p=128),
                        pav[:, :, 0, :])
                    nc.vector.tensor_copy(
                        xr[:].rearrange("u (h p) -> u h p", p=128),
                        pav[:, :, 1, :])

                    ps_b = pb.tile([128, 512], f32, tag="pb")
                    rhs_xi = [
                        mt[:, 512:2304].rearrange(
                            "u (a v) -> u a v", v=896)[:, :, 0:256],
                        mt[:, 1024:2304].rearrange(
                            "u (a v) -> u a v", v=640)[:, :, 0:256],
                    ]
                    for h in (0, 1):
                        hb = slice(h * 128, (h + 1) * 128)
                        nc.tensor.matmul(ps_b[:], xr[:, hb],
                                         mt[:, 256 + h * 512 : 768 + h * 512],
                                         start=(h == 0), stop=False)
                        nc.tensor.matmul(ps_b[:], xi[:, hb], rhs_xi[h],
                                         start=False, stop=(h == 1))
                    if l < N_LAYERS - 1:
                        nc.scalar.copy(s_r[0:64, sb], ps_b[0:64, 256:512])
                        nc.scalar.copy(s_r[64:128, sb],
                                       ps_b[64:128, 511:255:-1])
                        nc.vector.tensor_copy(s_i[0:64, sb], ps_b[0:64, 0:256])
                        nc.vector.tensor_copy(s_i[64:128, sb],
                                              ps_b[64:128, 255::-1])
                    else:
                        # psum cols: (i par0 | i par1 | r par0 | r par1)
                        scr = fp.tile([128, 512], f32, tag="scr")
                        for k in range(4):
                            comp, par = divmod(k, 2)
                            col = b * 4 + comp * 2 + par
                            nc.scalar.activation(
                                scr[:, k * 128 : (k + 1) * 128],
                                ps_b[:, k * 128 : (k + 1) * 128], Square,
                                accum_out=rows[:, col : col + 1])

            ps_f = pf.tile([1, 2 * NB], f32, tag="pf")
            nc.tensor.matmul(ps_f[:], spv[:, 0:1], rows[:, 0 : 4 * NB : 2],
                             start=True, stop=False)
            nc.tensor.matmul(ps_f[:], spv[:, 1:2], rows[:, 1 : 4 * NB : 2],
                             start=False, stop=True)
            res_t = msc.tile([1, 2 * NB], f32, tag="rest")
            nc.vector.tensor_copy(res_t[:], ps_f[:])
            nc.vector.tensor_add(res_s[:], res_t[0:1, 0 : 2 * NB : 2],
                                 res_t[0:1, 1 : 2 * NB : 2])
            nc.sync.dma_start(res_d[:], res_s[:])

    nc.compile()
    return nc


def _get_nc():
    key = A_DT + B_DT + str(NEG_ON_POOL)
    if key not in _CACHE:
        _CACHE[key] = _build_module()
    return _CACHE[key]


# ----------------------------------------------------------------- interface


def _run(x, thetas, trace=False):
    from concourse.bass_utils import run_bass_kernel_spmd

    import ml_dtypes

    x = np.asarray(x, dtype=np.float32)
    thetas = np.asarray(thetas, dtype=np.float32)
    np_map = {"bf16": ml_dtypes.bfloat16, "f16": np.float16,
              "f32r": np.float32, "f32": np.float32}
    np_a = np_map[A_DT]
    np_b = np_map[B_DT]
    pc = lambda v: np.array([bin(int(i)).count("1") for i in v])
    sp = (1.0 - 2.0 * (pc(np.arange(128)) % 2)).astype(np.float32)
    spv = np.stack([sp, -sp], axis=1)
    one = np.ones((1, NB), dtype=np_a)
    in_maps = []
    for c in range(N_CORES):
        ma, mb = _host_inputs_for_core(x[c * NB : (c + 1) * NB], thetas)
        if A_DT == B_DT:
            in_maps.append({"mab": np.concatenate([ma, mb], axis=2).astype(np_a),
                            "spv": spv, "one": one})
        else:
            in_maps.append({"ma": ma.astype(np_a), "mb": mb.astype(np_b),
                            "spv": spv, "one": one})
    nc = _get_nc()
    try:
        r = run_bass_kernel_spmd(nc, in_maps, core_ids=list(range(N_CORES)),
                                 trace=trace)
    except ModuleNotFoundError:
        r = run_bass_kernel_spmd(nc, in_maps, core_ids=list(range(N_CORES)),
                                 trace=False)
    out = np.concatenate([r.results[c]["res"].reshape(NB) for c in range(N_CORES)])
    return out.astype(np.float32), r


def kernel(x, thetas):
    out, _ = _run(x, thetas, trace=False)
    return out


# revision 13
# speedup vs baseline: 1.2039x; 1.2039x over previous
"""Trainium2 Bass kernel for the 15-qubit, 4-layer variational circuit.

Problem: batch of 32 circuit evaluations; each evolves a 2^15 state through
4 layers of (RY RZ RX RZ RX per qubit + CNOT chain) and measures <Z...Z>.

Strategy (8 NeuronCores, batch-parallel, zero collectives):
  - Each core simulates 4 batch elements; the full state (4 x 32768 complex64
    as separate f32 re/im planes) lives in SBUF the whole time.
  - State layout per batch: S[p, f], p = 7 "partition qubits" (6,0,1,2,3,4,5
    MSB->LSB), f = 8 "free qubits" (7..14 MSB->LSB).
  - Per layer, two PE matmul stages, each of which applies a dense fused gate
    matrix AND transposes the layout (out = lhsT.T @ rhs with the state as the
    stationary operand):
      stage A: A = C_P @ kron(G_q for partition qubits)   (128x128 complex)
      stage B: K = C_F @ kron(G_q for free qubits)        (256x256 complex)
    where G_q = RX RZ RX RZ(x2) RY(x1) is the per-qubit fused 1q gate and
    C_P/C_F are the intra-group CNOT chains folded in host-side.
  - The one straddling CNOT(6,7) conjugated past C_F becomes
    "if q6(p): f ^= 0xFF" (free-index reversal), folded into the PSUM->SBUF
    eviction copy after stage B at zero cost. On the last layer it is skipped
    entirely (XOR by 0xFF has even parity, so the Z...Z sign is unchanged).
  - Real/imag cross terms are packed side by side in the moving operand so
    every matmul has free dim >= 256, where float32r runs at full PE rate.
  - Finale: ACT squares |amp|^2 straight out of PSUM, DVE applies the
    (-1)^popcount sign tile, ACT row-reduces via accum_out, one tiny matmul
    reduces over partitions, and a [1,4] DMA returns the 4 expectations.
"""

import sys

if "/opt/trn_rl_repo" not in sys.path:
    sys.path.append("/opt/trn_rl_repo")

import numpy as np

N_QUBITS = 15
N_LAYERS = 4
BATCH = 32
DIM = 1 << N_QUBITS
N_CORES = 8
NB = BATCH // N_CORES  # batches per core

PART_QUBITS = [6, 0, 1, 2, 3, 4, 5]       # p bit MSB->LSB (q6 = p MSB)
FREE_QUBITS = [7, 8, 9, 10, 11, 12, 13, 14]  # f bit MSB->LSB

A_DT = "f16"   # stage-A matmul dtype: f32r | f16 | bf16 | f32
B_DT = "f16"   # stage-B matmul dtype (merged DMA when equal to A_DT)
NEG_ON_POOL = True

# ----------------------------------------------------------------- host math


def _rx(t):
    c, s = np.cos(t / 2), -1j * np.sin(t / 2)
    return np.array([[c, s], [s, c]], dtype=np.complex128)


def _ry(t):
    c, s = np.cos(t / 2), np.sin(t / 2)
    return np.array([[c, -s], [s, c]], dtype=np.complex128)


def _rz(t):
    return np.array(
        [[np.exp(-1j * t / 2), 0], [0, np.exp(1j * t / 2)]], dtype=np.complex128
    )


def _chain_perm(qubit_list, nbits, bitpos):
    """perm[old] = new index after CNOT(q, q+1) for q in qubit_list."""
    idx = np.arange(1 << nbits)
    bits = {q: (idx >> (nbits - 1 - pos)) & 1 for q, pos in bitpos.items()}
    for q in qubit_list:
        bits[q + 1] = bits[q + 1] ^ bits[q]
    new = np.zeros(1 << nbits, dtype=np.int64)
    for q, pos in bitpos.items():
        new |= bits[q] << (nbits - 1 - pos)
    return new


def _kron_list(mats):
    out = np.array([[1.0 + 0j]])
    for m in mats:
        out = np.kron(out, m)
    return out


_P_BITPOS = {q: i for i, q in enumerate(PART_QUBITS)}
_F_BITPOS = {q: i for i, q in enumerate(FREE_QUBITS)}
_PERM_P = _chain_perm(range(0, 6), 7, _P_BITPOS)
_PERM_F = _chain_perm(range(7, 14), 8, _F_BITPOS)


def _stage_matrices(x_b, thetas):
    x1 = np.arcsin(np.float64(x_b))
    x2 = np.arccos(np.float64(x_b) ** 2)
    E = _rz(x2) @ _ry(x1)
    As, Ks = [], []
    for l in range(N_LAYERS):
        G = {}
        for q in range(N_QUBITS):
            th = thetas[l, q].astype(np.float64)
            G[q] = _rx(th[2]) @ _rz(th[1]) @ _rx(th[0]) @ E
        kp = _kron_list([G[q] for q in PART_QUBITS])
        A = np.zeros_like(kp)
        A[_PERM_P, :] = kp
        kf = _kron_list([G[q] for q in FREE_QUBITS])
        K = np.zeros_like(kf)
        K[_PERM_F, :] = kf
        if l == N_LAYERS - 1:
            # regroup output index by popcount parity: j -> (parity, j & 127),
            # so the finale reduces sign-homogeneous contiguous blocks
            j = np.arange(256)
            par = np.array([bin(v).count("1") & 1 for v in j])
            jprime = (par << 7) | (j & 127)
            K2 = np.zeros_like(K)
            K2[jprime, :] = K
            K = K2
        As.append(A)
        Ks.append(K)
    return As, Ks


def _sign_tile():
    pc = lambda v: np.array([bin(int(i)).count("1") for i in v])
    sp = 1.0 - 2.0 * (pc(np.arange(128)) % 2)
    sf = 1.0 - 2.0 * (pc(np.arange(256)) % 2)
    return (sp[:, None] * sf[None, :]).astype(np.float32)


def _host_inputs_for_core(x_core, thetas):
    """MA [NB*4, 128, 384] and MB [NB*4, 2, 128, 768] f32 for one core."""
    ma = np.zeros((NB * N_LAYERS, 128, 256), dtype=np.float32)
    mb = np.zeros((NB * N_LAYERS, 128, 1024), dtype=np.float32)
    # [Ai | Ar | Ki0 | Kr0 | Ki1 | Kr1] per (b, l)
    for b in range(NB):
        As, Ks = _stage_matrices(x_core[b], thetas)
        for l in range(N_LAYERS):
            A = As[l]
            ArT = np.ascontiguousarray(A.real.T).astype(np.float32)
            AiT = np.ascontiguousarray(A.imag.T).astype(np.float32)
            ma[b * N_LAYERS + l] = np.concatenate([AiT, ArT], axis=1)
            KT = Ks[l].T
            KTr = KT.real.astype(np.float32)
            KTi = KT.imag.astype(np.float32)
            mb[b * N_LAYERS + l] = np.concatenate(
                [KTi[0:128], KTr[0:128], KTi[128:256], KTr[128:256]], axis=1)
    return ma, mb


# -------------------------------------------------------------- device build

_CACHE = {}


def _build_module():
    import concourse.bacc as bacc
    import concourse.mybir as mybir
    import concourse.tile as tile

    f32 = mybir.dt.float32
    dts = {"f32r": mybir.dt.float32r, "bf16": mybir.dt.bfloat16,
           "f16": mybir.dt.float16, "f32": mybir.dt.float32}
    dta = dts[A_DT]
    dtb = dts[B_DT]
    merged = A_DT == B_DT
    Square = mybir.ActivationFunctionType.Square
    Copy = mybir.ActivationFunctionType.Copy

    nc = bacc.Bacc("TRN2", target_bir_lowering=False, debug=False)
    if merged:
        mab_d = nc.dram_tensor("mab", [NB * N_LAYERS, 128, 1280], dta,
                               kind="ExternalInput")
    else:
        ma_d = nc.dram_tensor("ma", [NB * N_LAYERS, 128, 256], dta,
                              kind="ExternalInput")
        mb_d = nc.dram_tensor("mb", [NB * N_LAYERS, 128, 1024], dtb,
                              kind="ExternalInput")
    sp_d = nc.dram_tensor("spv", [128, 2], f32, kind="ExternalInput")
    x1_d = nc.dram_tensor("x1", [NB, 2, 1, 256], dtb, kind="ExternalInput")
    res_d = nc.dram_tensor("res", [1, NB], f32, kind="ExternalOutput")

    neg = (lambda o, i: nc.gpsimd.tensor_scalar_mul(o, i, -1.0)) \
        if NEG_ON_POOL else \
        (lambda o, i: nc.vector.tensor_scalar_mul(o, i, -1.0))

    with tile.TileContext(nc) as tc:
        with tc.tile_pool(name="state", bufs=1) as stp, \
             tc.tile_pool(name="xbuf", bufs=3) as xp, \
             tc.tile_pool(name="mats", bufs=6) as mp, \
             tc.tile_pool(name="fin", bufs=2) as fp, \
             tc.tile_pool(name="misc", bufs=1) as msc, \
             tc.tile_pool(name="pa", bufs=2, space="PSUM") as pa, \
             tc.tile_pool(name="pb", bufs=2, space="PSUM") as pb, \
             tc.tile_pool(name="pf", bufs=1, space="PSUM") as pf:

            s_r = stp.tile([128, 256 * NB], dta, tag="sr", name="sr")
            s_i = stp.tile([128, 256 * NB], dta, tag="si", name="si")
            spv = msc.tile([128, 2], f32, tag="spv")
            rows = msc.tile([128, 4 * NB], f32, tag="rows")
            res_s = msc.tile([1, NB], f32, tag="res")

            nc.sync.dma_start(spv[:], sp_d[:])
            nc.vector.memset(rows[:], 0.0)

            for l in range(N_LAYERS):
                for b in range(NB):
                    i_bl = b * N_LAYERS + l
                    # MT cols: [Ai 0:128 | Ar 128:256 | Ki0 256:512 |
                    #   Kr0 512:768 | Ki1 768:1024 | Kr1 1024:1280 |
                    #   nAi 1280:1408 | nKi0 1408:1664 | nKi1 1664:1920 | pad]
                    if merged:
                        mt = mp.tile([128, 2432], dta, tag="mt")
                        nc.sync.dma_start(mt[:, 0:1280], mab_d[i_bl])
                        mtb = mt[:, 256:1280]
                    else:
                        mt = mp.tile([128, 2432], dtb, tag="mt")
                        mta_t = mp.tile([128, 256], dta, tag="mta")
                        nc.sync.dma_start(mta_t[:], ma_d[i_bl])
                        nc.sync.dma_start(mt[:, 256:1280], mb_d[i_bl])
                    neg(mt[:, 1408:1664], mt[:, 256:512])
                    neg(mt[:, 1664:1920], mt[:, 768:1024])
                    if merged:
                        neg(mt[:, 1280:1408], mt[:, 0:128])
                        rhs_ar = mt[:, 0:256]
                        rhs_ai = mt[:, 128:2432].rearrange(
                            "u (a v) -> u a v", v=1152)[:, :, 0:128]
                    else:
                        mta_n = mp.tile([128, 384], dta, tag="mtan")
                        nc.vector.tensor_copy(mta_n[:, 0:256], mta_t[:])
                        neg(mta_n[:, 256:384], mta_t[:, 0:128])
                        rhs_ar = mta_n[:, 0:256]
                        rhs_ai = mta_n[:, 128:384]

                    sb = slice(b * 256, (b + 1) * 256)
                    xr = xp.tile([128, 256], dtb, tag="xr")
                    xi = xp.tile([128, 256], dtb, tag="xi")
                    if l == 0:
                        # |0...0> through stage A is just column 0 of A:
                        # only row u=0 (h=0 half) is nonzero
                        nc.vector.memset(xr[:], 0.0)
                        nc.vector.memset(xi[:], 0.0)
                        nc.sync.dma_start(xr[0:1, :], x1_d[b, 0])
                        nc.sync.dma_start(xi[0:1, :], x1_d[b, 1])
                    else:
                        ps_a = pa.tile([128, 512], f32, tag="pa")
                        for h in (0, 1):
                            sl = slice(b * 256 + h * 128,
                                       b * 256 + (h + 1) * 128)
                            po = slice(h * 256, (h + 1) * 256)
                            nc.tensor.matmul(ps_a[:, po], s_r[:, sl], rhs_ar,
                                             start=(h == 0), stop=False)
                            nc.tensor.matmul(ps_a[:, po], s_i[:, sl], rhs_ai,
                                             start=False, stop=(h == 1))
                        pav = ps_a[:].rearrange("u (h c p) -> u h c p",
                                                c=2, p=128)
                        nc.scalar.copy(
                            xi[:].rearrange("u (h p) -> u h p", p=128),
                            pav[:, :, 0, :])
                        nc.vector.tensor_copy(
                            xr[:].rearrange("u (h p) -> u h p", p=128),
                            pav[:, :, 1, :])

                    ps_b = pb.tile([128, 512], f32, tag="pb")
                    rhs_xi = [
                        mt[:, 512:2304].rearrange(
                            "u (a v) -> u a v", v=896)[:, :, 0:256],
                        mt[:, 1024:2304].rearrange(
                            "u (a v) -> u a v", v=640)[:, :, 0:256],
                    ]
                    for h in (0, 1):
                        hb = slice(h * 128, (h + 1) * 128)
                        nc.tensor.matmul(ps_b[:], xr[:, hb],
                                         mt[:, 256 + h * 512 : 768 + h * 512],
                                         start=(h == 0), stop=False)
                        nc.tensor.matmul(ps_b[:], xi[:, hb], rhs_xi[h],
                                         start=False, stop=(h == 1))
                    if l < N_LAYERS - 1:
                        nc.scalar.copy(s_r[0:64, sb], ps_b[0:64, 256:512])
                        nc.scalar.copy(s_r[64:128, sb],
                                       ps_b[64:128, 511:255:-1])
                        nc.vector.tensor_copy(s_i[0:64, sb], ps_b[0:64, 0:256])
                        nc.vector.tensor_copy(s_i[64:128, sb],
                                              ps_b[64:128, 255::-1])
                    else:
                        # psum cols: (i par0 | i par1 | r par0 | r par1)
                        scr = fp.tile([128, 512], f32, tag="scr")
                        for k in range(4):
                            comp, par = divmod(k, 2)
                            col = b * 4 + comp * 2 + par
                            nc.scalar.activation(
                                scr[:, k * 128 : (k + 1) * 128],
                                ps_b[:, k * 128 : (k + 1) * 128], Square,
                                accum_out=rows[:, col : col + 1])

            ps_f = pf.tile([1, 2 * NB], f32, tag="pf")
            nc.tensor.matmul(ps_f[:], spv[:, 0:1], rows[:, 0 : 4 * NB : 2],
                             start=True, stop=False)
            nc.tensor.matmul(ps_f[:], spv[:, 1:2], rows[:, 1 : 4 * NB : 2],
                             start=False, stop=True)
            res_t = msc.tile([1, 2 * NB], f32, tag="rest")
            nc.vector.tensor_copy(res_t[:], ps_f[:])
            nc.vector.tensor_add(res_s[:], res_t[0:1, 0 : 2 * NB : 2],
                                 res_t[0:1, 1 : 2 * NB : 2])
            nc.sync.dma_start(res_d[:], res_s[:])

    nc.compile()
    return nc


def _get_nc():
    key = A_DT + B_DT + str(NEG_ON_POOL)
    if key not in _CACHE:
        _CACHE[key] = _build_module()
    return _CACHE[key]


# ----------------------------------------------------------------- interface


def _run(x, thetas, trace=False):
    from concourse.bass_utils import run_bass_kernel_spmd

    import ml_dtypes

    x = np.asarray(x, dtype=np.float32)
    thetas = np.asarray(thetas, dtype=np.float32)
    np_map = {"bf16": ml_dtypes.bfloat16, "f16": np.float16,
              "f32r": np.float32, "f32": np.float32}
    np_a = np_map[A_DT]
    np_b = np_map[B_DT]
    pc = lambda v: np.array([bin(int(i)).count("1") for i in v])
    sp = (1.0 - 2.0 * (pc(np.arange(128)) % 2)).astype(np.float32)
    spv = np.stack([sp, -sp], axis=1)
    in_maps = []
    for c in range(N_CORES):
        ma, mb = _host_inputs_for_core(x[c * NB : (c + 1) * NB], thetas)
        x1 = np.zeros((NB, 2, 1, 256), dtype=np.float32)
        for b in range(NB):
            As, _ = _stage_matrices(x[c * NB + b], thetas)
            col = As[0][:, 0]
            x1[b, 0, 0, 0:128] = col.real
            x1[b, 1, 0, 0:128] = col.imag
        if A_DT == B_DT:
            in_maps.append({"mab": np.concatenate([ma, mb], axis=2).astype(np_a),
                            "spv": spv, "x1": x1.astype(np_b)})
        else:
            in_maps.append({"ma": ma.astype(np_a), "mb": mb.astype(np_b),
                            "spv": spv, "x1": x1.astype(np_b)})
    nc = _get_nc()
    try:
        r = run_bass_kernel_spmd(nc, in_maps, core_ids=list(range(N_CORES)),
                                 trace=trace)
    except ModuleNotFoundError:
        r = run_bass_kernel_spmd(nc, in_maps, core_ids=list(range(N_CORES)),
                                 trace=False)
    out = np.concatenate([r.results[c]["res"].reshape(NB) for c in range(N_CORES)])
    return out.astype(np.float32), r


def kernel(x, thetas):
    out, _ = _run(x, thetas, trace=False)
    return out


# revision 15
# speedup vs baseline: 1.2221x; 1.0152x over previous
"""Trainium2 Bass kernel for the 15-qubit, 4-layer variational circuit.

Problem: batch of 32 circuit evaluations; each evolves a 2^15 state through
4 layers of (RY RZ RX RZ RX per qubit + CNOT chain) and measures <Z...Z>.

Strategy (8 NeuronCores, batch-parallel, zero collectives):
  - Each core simulates 4 batch elements; the full state (4 x 32768 complex64
    as separate f32 re/im planes) lives in SBUF the whole time.
  - State layout per batch: S[p, f], p = 7 "partition qubits" (6,0,1,2,3,4,5
    MSB->LSB), f = 8 "free qubits" (7..14 MSB->LSB).
  - Per layer, two PE matmul stages, each of which applies a dense fused gate
    matrix AND transposes the layout (out = lhsT.T @ rhs with the state as the
    stationary operand):
      stage A: A = C_P @ kron(G_q for partition qubits)   (128x128 complex)
      stage B: K = C_F @ kron(G_q for free qubits)        (256x256 complex)
    where G_q = RX RZ RX RZ(x2) RY(x1) is the per-qubit fused 1q gate and
    C_P/C_F are the intra-group CNOT chains folded in host-side.
  - The one straddling CNOT(6,7) conjugated past C_F becomes
    "if q6(p): f ^= 0xFF" (free-index reversal), folded into the PSUM->SBUF
    eviction copy after stage B at zero cost. On the last layer it is skipped
    entirely (XOR by 0xFF has even parity, so the Z...Z sign is unchanged).
  - Real/imag cross terms are packed side by side in the moving operand so
    every matmul has free dim >= 256, where float32r runs at full PE rate.
  - Finale: ACT squares |amp|^2 straight out of PSUM, DVE applies the
    (-1)^popcount sign tile, ACT row-reduces via accum_out, one tiny matmul
    reduces over partitions, and a [1,4] DMA returns the 4 expectations.
"""

import sys

if "/opt/trn_rl_repo" not in sys.path:
    sys.path.append("/opt/trn_rl_repo")

import numpy as np

N_QUBITS = 15
N_LAYERS = 4
BATCH = 32
DIM = 1 << N_QUBITS
N_CORES = 8
NB = BATCH // N_CORES  # batches per core

PART_QUBITS = [6, 0, 1, 2, 3, 4, 5]       # p bit MSB->LSB (q6 = p MSB)
FREE_QUBITS = [7, 8, 9, 10, 11, 12, 13, 14]  # f bit MSB->LSB

A_DT = "f16"   # stage-A matmul dtype: f32r | f16 | bf16 | f32
B_DT = "f16"   # stage-B matmul dtype (merged DMA when equal to A_DT)
NEG_ON_POOL = True

# ----------------------------------------------------------------- host math


def _rx(t):
    c, s = np.cos(t / 2), -1j * np.sin(t / 2)
    return np.array([[c, s], [s, c]], dtype=np.complex128)


def _ry(t):
    c, s = np.cos(t / 2), np.sin(t / 2)
    return np.array([[c, -s], [s, c]], dtype=np.complex128)


def _rz(t):
    return np.array(
        [[np.exp(-1j * t / 2), 0], [0, np.exp(1j * t / 2)]], dtype=np.complex128
    )


def _chain_perm(qubit_list, nbits, bitpos):
    """perm[old] = new index after CNOT(q, q+1) for q in qubit_list."""
    idx = np.arange(1 << nbits)
    bits = {q: (idx >> (nbits - 1 - pos)) & 1 for q, pos in bitpos.items()}
    for q in qubit_list:
        bits[q + 1] = bits[q + 1] ^ bits[q]
    new = np.zeros(1 << nbits, dtype=np.int64)
    for q, pos in bitpos.items():
        new |= bits[q] << (nbits - 1 - pos)
    return new


def _kron_list(mats):
    out = np.array([[1.0 + 0j]])
    for m in mats:
        out = np.kron(out, m)
    return out


_P_BITPOS = {q: i for i, q in enumerate(PART_QUBITS)}
_F_BITPOS = {q: i for i, q in enumerate(FREE_QUBITS)}
_PERM_P = _chain_perm(range(0, 6), 7, _P_BITPOS)
_PERM_F = _chain_perm(range(7, 14), 8, _F_BITPOS)


def _stage_matrices(x_b, thetas):
    x1 = np.arcsin(np.float64(x_b))
    x2 = np.arccos(np.float64(x_b) ** 2)
    E = _rz(x2) @ _ry(x1)
    As, Ks = [], []
    for l in range(N_LAYERS):
        G = {}
        for q in range(N_QUBITS):
            th = thetas[l, q].astype(np.float64)
            G[q] = _rx(th[2]) @ _rz(th[1]) @ _rx(th[0]) @ E
        kp = _kron_list([G[q] for q in PART_QUBITS])
        A = np.zeros_like(kp)
        A[_PERM_P, :] = kp
        kf = _kron_list([G[q] for q in FREE_QUBITS])
        K = np.zeros_like(kf)
        K[_PERM_F, :] = kf
        if l == N_LAYERS - 1:
            # regroup output index by popcount parity: j -> (parity, j & 127),
            # so the finale reduces sign-homogeneous contiguous blocks
            j = np.arange(256)
            par = np.array([bin(v).count("1") & 1 for v in j])
            jprime = (par << 7) | (j & 127)
            K2 = np.zeros_like(K)
            K2[jprime, :] = K
            K = K2
        As.append(A)
        Ks.append(K)
    return As, Ks


def _sign_tile():
    pc = lambda v: np.array([bin(int(i)).count("1") for i in v])
    sp = 1.0 - 2.0 * (pc(np.arange(128)) % 2)
    sf = 1.0 - 2.0 * (pc(np.arange(256)) % 2)
    return (sp[:, None] * sf[None, :]).astype(np.float32)


def _host_inputs_for_core(x_core, thetas):
    """MA [NB*4, 128, 384] and MB [NB*4, 2, 128, 768] f32 for one core."""
    ma = np.zeros((NB * N_LAYERS, 128, 256), dtype=np.float32)
    mb = np.zeros((NB * N_LAYERS, 128, 1024), dtype=np.float32)
    # [Ai | Ar | Ki0 | Kr0 | Ki1 | Kr1] per (b, l)
    for b in range(NB):
        As, Ks = _stage_matrices(x_core[b], thetas)
        for l in range(N_LAYERS):
            A = As[l]
            ArT = np.ascontiguousarray(A.real.T).astype(np.float32)
            AiT = np.ascontiguousarray(A.imag.T).astype(np.float32)
            ma[b * N_LAYERS + l] = np.concatenate([AiT, ArT], axis=1)
            KT = Ks[l].T
            KTr = KT.real.astype(np.float32)
            KTi = KT.imag.astype(np.float32)
            mb[b * N_LAYERS + l] = np.concatenate(
                [KTi[0:128], KTr[0:128], KTi[128:256], KTr[128:256]], axis=1)
    return ma, mb


# -------------------------------------------------------------- device build

_CACHE = {}


def _build_module():
    import concourse.bacc as bacc
    import concourse.mybir as mybir
    import concourse.tile as tile

    f32 = mybir.dt.float32
    dts = {"f32r": mybir.dt.float32r, "bf16": mybir.dt.bfloat16,
           "f16": mybir.dt.float16, "f32": mybir.dt.float32}
    dta = dts[A_DT]
    dtb = dts[B_DT]
    merged = A_DT == B_DT
    Square = mybir.ActivationFunctionType.Square
    Copy = mybir.ActivationFunctionType.Copy

    nc = bacc.Bacc("TRN2", target_bir_lowering=False, debug=False)
    if merged:
        mab_d = nc.dram_tensor("mab", [NB * N_LAYERS, 128, 1280], dta,
                               kind="ExternalInput")
    else:
        ma_d = nc.dram_tensor("ma", [NB * N_LAYERS, 128, 256], dta,
                              kind="ExternalInput")
        mb_d = nc.dram_tensor("mb", [NB * N_LAYERS, 128, 1024], dtb,
                              kind="ExternalInput")
    sp_d = nc.dram_tensor("spv", [128, 2], f32, kind="ExternalInput")
    x1_d = nc.dram_tensor("x1", [NB, 2, 1, 256], dtb, kind="ExternalInput")
    res_d = nc.dram_tensor("res", [1, NB], f32, kind="ExternalOutput")

    neg = (lambda o, i: nc.gpsimd.tensor_scalar_mul(o, i, -1.0)) \
        if NEG_ON_POOL else \
        (lambda o, i: nc.vector.tensor_scalar_mul(o, i, -1.0))

    with tile.TileContext(nc) as tc:
        with tc.tile_pool(name="state", bufs=1) as stp, \
             tc.tile_pool(name="xbuf", bufs=3) as xp, \
             tc.tile_pool(name="mats", bufs=8) as mp, \
             tc.tile_pool(name="fin", bufs=2) as fp, \
             tc.tile_pool(name="misc", bufs=1) as msc, \
             tc.tile_pool(name="pa", bufs=2, space="PSUM") as pa, \
             tc.tile_pool(name="pb", bufs=2, space="PSUM") as pb, \
             tc.tile_pool(name="pf", bufs=1, space="PSUM") as pf:

            s_r = stp.tile([128, 256 * NB], dta, tag="sr", name="sr")
            s_i = stp.tile([128, 256 * NB], dta, tag="si", name="si")
            spv = msc.tile([128, 2], f32, tag="spv")
            rows = msc.tile([128, 4 * NB], f32, tag="rows")
            res_s = msc.tile([1, NB], f32, tag="res")

            nc.vector.memset(rows[:], 0.0)
            ps_f = pf.tile([1, 2 * NB], f32, tag="pf")
            nc.sync.dma_start(spv[:], sp_d[:])
            x1s = msc.tile([1, 2048], dtb, tag="x1s")
            nc.sync.dma_start(x1s[:], x1_d[:].rearrange("b c o f -> o (b c f)"))

            for l in range(N_LAYERS):
                for b in range(NB):
                    i_bl = b * N_LAYERS + l
                    # MT cols: [Ai 0:128 | Ar 128:256 | Ki0 256:512 |
                    #   Kr0 512:768 | Ki1 768:1024 | Kr1 1024:1280 |
                    #   nAi 1280:1408 | nKi0 1408:1664 | nKi1 1664:1920 | pad]
                    if merged:
                        mt = mp.tile([128, 2432], dta, tag="mt")
                        nc.sync.dma_start(mt[:, 0:1280], mab_d[i_bl])
                        mtb = mt[:, 256:1280]
                    else:
                        mt = mp.tile([128, 2432], dtb, tag="mt")
                        mta_t = mp.tile([128, 256], dta, tag="mta")
                        nc.sync.dma_start(mta_t[:], ma_d[i_bl])
                        nc.sync.dma_start(mt[:, 256:1280], mb_d[i_bl])
                    neg(mt[:, 1408:1664], mt[:, 256:512])
                    neg(mt[:, 1664:1920], mt[:, 768:1024])
                    if merged:
                        if l > 0:
                            neg(mt[:, 1280:1408], mt[:, 0:128])
                        rhs_ar = mt[:, 0:256]
                        rhs_ai = mt[:, 128:2432].rearrange(
                            "u (a v) -> u a v", v=1152)[:, :, 0:128]
                    else:
                        mta_n = mp.tile([128, 384], dta, tag="mtan")
                        nc.vector.tensor_copy(mta_n[:, 0:256], mta_t[:])
                        neg(mta_n[:, 256:384], mta_t[:, 0:128])
                        rhs_ar = mta_n[:, 0:256]
                        rhs_ai = mta_n[:, 128:384]

                    sb = slice(b * 256, (b + 1) * 256)
                    xr = xp.tile([128, 256], dtb, tag="xr")
                    xi = xp.tile([128, 256], dtb, tag="xi")
                    if l == 0:
                        # |0...0> through stage A is just column 0 of A:
                        # only row u=0 (h=0 half) is nonzero
                        nc.vector.memset(xr[:], 0.0)
                        nc.vector.memset(xi[:], 0.0)
                        nc.vector.tensor_copy(
                            xr[0:1, :], x1s[0:1, b * 512 : b * 512 + 256])
                        nc.vector.tensor_copy(
                            xi[0:1, :], x1s[0:1, b * 512 + 256 : b * 512 + 512])
                    else:
                        ps_a = pa.tile([128, 512], f32, tag="pa")
                        for h in (0, 1):
                            sl = slice(b * 256 + h * 128,
                                       b * 256 + (h + 1) * 128)
                            po = slice(h * 256, (h + 1) * 256)
                            nc.tensor.matmul(ps_a[:, po], s_r[:, sl], rhs_ar,
                                             start=(h == 0), stop=False)
                            nc.tensor.matmul(ps_a[:, po], s_i[:, sl], rhs_ai,
                                             start=False, stop=(h == 1))
                        pav = ps_a[:].rearrange("u (h c p) -> u h c p",
                                                c=2, p=128)
                        nc.scalar.copy(
                            xi[:].rearrange("u (h p) -> u h p", p=128),
                            pav[:, :, 0, :])
                        nc.vector.tensor_copy(
                            xr[:].rearrange("u (h p) -> u h p", p=128),
                            pav[:, :, 1, :])

                    ps_b = pb.tile([128, 512], f32, tag="pb")
                    rhs_xi = [
                        mt[:, 512:2304].rearrange(
                            "u (a v) -> u a v", v=896)[:, :, 0:256],
                        mt[:, 1024:2304].rearrange(
                            "u (a v) -> u a v", v=640)[:, :, 0:256],
                    ]
                    for h in (0, 1):
                        nc.tensor.matmul(ps_b[:], xr[:, h * 128 : h * 128 + 128],
                                         mt[:, 256 + h * 512 : 768 + h * 512],
                                         start=(h == 0), stop=False)
                    for h in (0, 1):
                        nc.tensor.matmul(ps_b[:], xi[:, h * 128 : h * 128 + 128],
                                         rhs_xi[h], start=False, stop=(h == 1))
                    if l < N_LAYERS - 1:
                        nc.scalar.copy(s_r[0:64, sb], ps_b[0:64, 256:512])
                        nc.scalar.copy(s_r[64:128, sb],
                                       ps_b[64:128, 511:255:-1])
                        nc.vector.tensor_copy(s_i[0:64, sb], ps_b[0:64, 0:256])
                        nc.vector.tensor_copy(s_i[64:128, sb],
                                              ps_b[64:128, 255::-1])
                    else:
                        # psum cols: (i par0 | i par1 | r par0 | r par1)
                        scr = fp.tile([128, 512], f32, tag="scr")
                        for k in range(4):
                            comp, par = divmod(k, 2)
                            col = b * 4 + comp * 2 + par
                            nc.scalar.activation(
                                scr[:, k * 128 : (k + 1) * 128],
                                ps_b[:, k * 128 : (k + 1) * 128], Square,
                                accum_out=rows[:, col : col + 1])


            nc.tensor.matmul(ps_f[:], spv[:, 0:1], rows[:, 0 : 4 * NB : 2],
                             start=True, stop=False)
            nc.tensor.matmul(ps_f[:], spv[:, 1:2], rows[:, 1 : 4 * NB : 2],
                             start=False, stop=True)
            res_t = msc.tile([1, 2 * NB], f32, tag="rest")
            nc.vector.tensor_copy(res_t[:], ps_f[:])
            nc.vector.tensor_add(res_s[:], res_t[0:1, 0 : 2 * NB : 2],
                                 res_t[0:1, 1 : 2 * NB : 2])
            nc.sync.dma_start(res_d[:], res_s[:])

    nc.compile()
    return nc


def _get_nc():
    key = A_DT + B_DT + str(NEG_ON_POOL)
    if key not in _CACHE:
        _CACHE[key] = _build_module()
    return _CACHE[key]


# ----------------------------------------------------------------- interface


def _run(x, thetas, trace=False):
    from concourse.bass_utils import run_bass_kernel_spmd

    import ml_dtypes

    x = np.asarray(x, dtype=np.float32)
    thetas = np.asarray(thetas, dtype=np.float32)
    np_map = {"bf16": ml_dtypes.bfloat16, "f16": np.float16,
              "f32r": np.float32, "f32": np.float32}
    np_a = np_map[A_DT]
    np_b = np_map[B_DT]
    pc = lambda v: np.array([bin(int(i)).count("1") for i in v])
    sp = (1.0 - 2.0 * (pc(np.arange(128)) % 2)).astype(np.float32)
    spv = np.stack([sp, -sp], axis=1)
    in_maps = []
    for c in range(N_CORES):
        ma, mb = _host_inputs_for_core(x[c * NB : (c + 1) * NB], thetas)
        x1 = np.zeros((NB, 2, 1, 256), dtype=np.float32)
        for b in range(NB):
            As, _ = _stage_matrices(x[c * NB + b], thetas)
            col = As[0][:, 0]
            x1[b, 0, 0, 0:128] = col.real
            x1[b, 1, 0, 0:128] = col.imag
        if A_DT == B_DT:
            in_maps.append({"mab": np.concatenate([ma, mb], axis=2).astype(np_a),
                            "spv": spv, "x1": x1.astype(np_b)})
        else:
            in_maps.append({"ma": ma.astype(np_a), "mb": mb.astype(np_b),
                            "spv": spv, "x1": x1.astype(np_b)})
    nc = _get_nc()
    try:
        r = run_bass_kernel_spmd(nc, in_maps, core_ids=list(range(N_CORES)),
                                 trace=trace)
    except ModuleNotFoundError:
        r = run_bass_kernel_spmd(nc, in_maps, core_ids=list(range(N_CORES)),
                                 trace=False)
    out = np.concatenate([r.results[c]["res"].reshape(NB) for c in range(N_CORES)])
    return out.astype(np.float32), r


def kernel(x, thetas):
    out, _ = _run(x, thetas, trace=False)
    return out
